# revision 36
# baseline (speedup 1.0000x reference)
import sys

import numpy as np

if "/opt/trn_rl_repo" not in sys.path:
    sys.path.insert(0, "/opt/trn_rl_repo")

import concourse.bacc as bacc
import concourse.mybir as mybir
import concourse.tile as tile
from concourse.bass_utils import run_bass_kernel_spmd

# Problem constants (hardcoded per harness contract)
B, C, K = 32768, 1000, 5
N_CORES = 8
ROWS = B // N_CORES          # 4096 rows per core
P = 128                      # partitions
NT = ROWS // P               # 32 row-slots per partition
# Wave sizes (rows per partition per wave). Small leading waves so the
# scalar engine starts exp as early as possible; small final wave so the
# tail after the last DMA is short.
TBS = [4, 4, 4, 4, 4, 4, 4, 2, 1, 1]
assert sum(TBS) == NT
# DMA ring per wave: 0=sync HWDGE, 1=gpsimd SWDGE. Strict alternation so
# delivery order matches consumption order (each ring is FIFO and pays a
# fixed inter-DMA gap, so delivery time barely grows with wave size —
# only the first consumed waves need to be small).
RING = [0, 1, 0, 1, 0, 1, 0, 1, 0, 1]
FP32 = mybir.dt.float32
FP16 = mybir.dt.float16


def _build_kernel():
    nc = bacc.Bacc()
    x = nc.declare_dram_parameter("x", [P, NT * C], FP16, isOutput=False)
    glog = nc.declare_dram_parameter("glog", [P, NT * K], FP16, isOutput=False)
    out = nc.declare_dram_parameter("out", [1, 1], FP32, isOutput=True)

    with tile.TileContext(nc) as tc:
        from contextlib import ExitStack
        with ExitStack() as stack:
            wp = stack.enter_context(tc.tile_pool(name="wave", bufs=1))
            fp = stack.enter_context(tc.tile_pool(name="fold", bufs=2))
            pp = stack.enter_context(tc.tile_pool(name="persist", bufs=1))

            g_sb = pp.tile([P, NT * K], FP16)
            denom = pp.tile([P, NT], FP32)
            numer = pp.tile([P, NT], FP32)
            rec = pp.tile([P, NT], FP32)
            loss = pp.tile([P, NT], FP32)
            lsum_a = pp.tile([P, 1], FP32)
            scratch = pp.tile([P, 1], FP32)

            # Warm the exp table while the first DMAs are in flight.
            nc.scalar.memzero(scratch[:])
            nc.scalar.activation(
                out=scratch[:], in_=scratch[:],
                func=mybir.ActivationFunctionType.Exp,
            )

            # Pre-create wave tiles so scalar-ring DMAs can be issued first.
            wts = []
            off = 0
            for wi, tb in enumerate(TBS):
                wts.append(wp.tile([P, tb * C], FP16, name=f"wt{wi}"))
            # glog first on the scalar ring (its exp gates the numerators),
            # then the scalar-ring wave DMAs — all issued while the scalar
            # engine is idle waiting for the first wave.
            nc.scalar.dma_start(out=g_sb[:], in_=glog[:])
            off = 0
            for wi, tb in enumerate(TBS):
                if RING[wi] == 2:
                    nc.scalar.dma_start(
                        out=wts[wi][:], in_=x[:, off * C:off * C + tb * C],
                    )
                off += tb
            # Numerator: exp of host-gathered logits, reduce per row-slot.
            nc.scalar.activation(
                out=g_sb[:], in_=g_sb[:], func=mybir.ActivationFunctionType.Exp,
            )
            nc.vector.tensor_reduce(
                out=numer[:],
                in_=g_sb[:].rearrange("p (t k) -> p t k", k=K),
                axis=mybir.AxisListType.X,
                op=mybir.AluOpType.add,
            )

            off = 0
            for wi, tb in enumerate(TBS):
                n = tb * C
                wt = wts[wi]
                if RING[wi] != 2:
                    eng = nc.sync if RING[wi] == 0 else nc.gpsimd
                    eng.dma_start(out=wt[:], in_=x[:, off * C:off * C + n])
                if tb == 1 and wi >= len(TBS) - 2:
                    # Tail waves (one row per partition): the ACT
                    # accumulator IS the row denominator — no DVE folds,
                    # so the post-last-ACT chain is minimal.
                    nc.scalar.activation(
                        out=wt[:], in_=wt[:],
                        func=mybir.ActivationFunctionType.Exp,
                        accum_out=denom[:, off:off + 1],
                    )
                else:
                    # exp in place, one ACTIVATE per wave
                    nc.scalar.activation(
                        out=wt[:], in_=wt[:],
                        func=mybir.ActivationFunctionType.Exp,
                    )
                    # Row sums via 2x-mode pairwise folds (fp16 TT) + TR.
                    w3 = wt[:].rearrange("p (t c) -> p t c", t=tb)
                    f1 = fp.tile([P, tb * 500], FP16)
                    f2 = fp.tile([P, tb * 250], FP16)
                    f3 = fp.tile([P, tb * 125], FP16)
                    f13 = f1[:].rearrange("p (t c) -> p t c", t=tb)
                    f23 = f2[:].rearrange("p (t c) -> p t c", t=tb)
                    f33 = f3[:].rearrange("p (t c) -> p t c", t=tb)
                    nc.vector.tensor_tensor(
                        out=f13, in0=w3[:, :, 0:500], in1=w3[:, :, 500:1000],
                        op=mybir.AluOpType.add,
                    )
                    nc.vector.tensor_tensor(
                        out=f23, in0=f13[:, :, 0:250], in1=f13[:, :, 250:500],
                        op=mybir.AluOpType.add,
                    )
                    nc.vector.tensor_tensor(
                        out=f33, in0=f23[:, :, 0:125], in1=f23[:, :, 125:250],
                        op=mybir.AluOpType.add,
                    )
                    nc.vector.tensor_reduce(
                        out=denom[:, off:off + tb], in_=f33,
                        axis=mybir.AxisListType.X, op=mybir.AluOpType.add,
                    )
                # Per-wave reciprocal + loss so the tail has almost nothing.
                nc.vector.reciprocal(
                    out=rec[:, off:off + tb], in_=denom[:, off:off + tb],
                )
                nc.vector.tensor_tensor(
                    out=loss[:, off:off + tb],
                    in0=numer[:, off:off + tb],
                    in1=rec[:, off:off + tb],
                    op=mybir.AluOpType.mult,
                )
                off += tb
                if wi == len(TBS) - 2:
                    # Partial loss sum over everything but the last wave,
                    # so the final chain after the last wave is minimal.
                    nc.vector.tensor_reduce(
                        out=lsum_a[:], in_=loss[:, 0:off],
                        axis=mybir.AxisListType.X, op=mybir.AluOpType.add,
                    )

            lsum = pp.tile([P, 1], FP32)
            total = pp.tile([1, 1], FP32)
            nc.vector.tensor_tensor(
                out=lsum[:], in0=lsum_a[:], in1=loss[:, NT - 1:NT],
                op=mybir.AluOpType.add,
            )
            nc.gpsimd.tensor_reduce(
                out=total[:], in_=lsum[:],
                axis=mybir.AxisListType.C, op=mybir.AluOpType.add,
            )
            nc.sync.dma_start(out=out[:], in_=total[:])

    if not nc.is_finalized():
        nc.finalize()
    return nc


_CACHE = {}


def _prep_inputs(outputs, complementary_labels):
    outputs = np.asarray(outputs, dtype=np.float32)
    labels = np.asarray(complementary_labels).astype(np.int64)

    in_maps = []
    for c in range(N_CORES):
        x_c = outputs[c * ROWS:(c + 1) * ROWS]
        lab = labels[c * ROWS:(c + 1) * ROWS]
        # Row assignment: row(wave wi, partition p, slot t) =
        #   P*off(wi) + p*tb + t   (off = cumulative TB before wave wi)
        x16 = np.empty((P, NT * C), dtype=np.float16)
        gl = np.empty((P, NT * K), dtype=np.float16)
        rows_of = np.empty((P, NT), dtype=np.int64)
        off = 0
        for tb in TBS:
            blk = np.arange(P * tb).reshape(P, tb)
            rows_of[:, off:off + tb] = P * off + blk
            off += tb
        # x16[p, j*C:(j+1)*C] = x_c[rows_of[p, j]]
        x16[:] = x_c[rows_of.reshape(-1)].reshape(P, NT * C).astype(np.float16)
        r = rows_of.reshape(-1)
        gl[:] = x_c[r[:, None], lab[r]].reshape(P, NT * K).astype(np.float16)
        in_maps.append({"x": x16, "glog": gl})
    return in_maps


def kernel(outputs, complementary_labels):
    if "nc" not in _CACHE:
        _CACHE["nc"] = _build_kernel()
    nc = _CACHE["nc"]
    in_maps = _prep_inputs(outputs, complementary_labels)
    res = run_bass_kernel_spmd(nc, in_maps, list(range(N_CORES)))
    total = 0.0
    for r in res.results:
        total += float(np.asarray(r["out"]).reshape(-1)[0])
    return np.array(total / B, dtype=np.float32)
